# revision 19
# baseline (speedup 1.0000x reference)
"""Multi-head causal attention (b=4, n=2048, d=1024, h=16) on 8 TRN2 cores.

Sharding: core c = (batch b = c//2, head-group g = c%2); each head-group is 8
heads = 512 of the 1024 model dims. QKV weights column-sharded, Wo row-sharded;
host sums the two head-group partial outputs per batch and adds the bias.

Per-core layout: everything is kept in "transposed" orientation so each
matmul feeds the next without any on-chip transposes:
  QT/KT [dout, tok] = W.T @ xT        (lhsT = W as stored, rhs = xT)
  scoresT [kv, q]   = KT_h.T @ QT_h   (contraction over head-dim, K=64,
                                       2 heads row-packed in the PE array)
  attnT             = exp(scoresT/8)  (ACT, PSUM->SBUF bf16; no max-subtraction:
                                       |scores/8| < ~2 for this input dist)
  causal mask       = DVE multiply by a constant lower-triangular bf16 tile
  ctxT [hd, q]      = V_h'.T @ attnT  (V_h' has a ones column appended, so PSUM
                                       row 64 accumulates the softmax denom)
  normalize         = DVE reciprocal_approx + gpsimd partition-broadcast +
                      DVE multiply at PSUM->SBUF copyback
  out [tok, dout]   = ctxT.T @ Wo     (partial over this head-group's 512 dims)

v2 emission schedule (vs ~303us baseline):
- software-pipelined kv loop: the ctx matmuls of kv-tile i-1 are emitted after
  the score-pair of tile i, and a deficit-paced filler queue (projection and
  out-proj units, each ~4 matmuls) is drained inside the loop so the PE always
  has independent work during the ACT-paced exp gaps.
- deficit model: ACT cost (2nw+352)/1.2 ns per tile vs PE cost ~0.63nw (score
  pair, measured 322ns@nw=512) + 0.83nw (ctx pair); the shortfall is pumped
  from the filler queue.
- V projections first + finer DMA interleave so attention starts ~11us in
  instead of ~25us.
- causal masks moved from gpsimd affine_select to DVE tri-tile multiplies
  (shorter latency, keeps gpsimd free for the normalize broadcasts).
- out-proj units are appended to the END of the filler queue as their chunk
  completes, so they back-fill the ACT-heavy final query chunk where no
  projection filler remains.
"""

import sys

if "/opt/trn_rl_repo" not in sys.path:
    sys.path.insert(0, "/opt/trn_rl_repo")

import numpy as np
import ml_dtypes

import concourse.bacc as bacc
import concourse.mybir as mybir
import concourse.tile as tile
from concourse import bass_utils

N_CORES = 8
B = 4          # batch
N = 2048       # sequence length
D = 1024       # model dim
H = 16         # total heads
HD = 64        # head dim
HH = 8         # heads per core
DH = 512       # model dims per core (HH * HD)
N_DT = 4       # 128-row d-tiles of DH (one head pair each)
N_QC = 4       # 512-wide query chunks
N_KT = 16      # 128-wide kv token tiles
N_TT = 16      # 128-wide token tiles
BF16 = mybir.dt.bfloat16
F32 = mybir.dt.float32
AF = mybir.ActivationFunctionType


def _emit(nc, tc, xt_d, wq_d, wk_d, wv_d, wo_d, out_d):
    import contextlib

    ctx = contextlib.ExitStack()
    with ctx:
        const = ctx.enter_context(tc.tile_pool(name="const", bufs=1))
        ps = ctx.enter_context(tc.tile_pool(name="ps", bufs=2, space="PSUM"))
        ctxp = ctx.enter_context(tc.tile_pool(name="ctxp", bufs=2, space="PSUM"))
        pop = ctx.enter_context(tc.tile_pool(name="pop", bufs=2, space="PSUM"))
        attn_pool = ctx.enter_context(tc.tile_pool(name="attn", bufs=12))
        small = ctx.enter_context(tc.tile_pool(name="small", bufs=3))
        outp = ctx.enter_context(tc.tile_pool(name="outp", bufs=4))

        # ---- input DMAs ----
        # wv + the first xT token-chunk first: the V projections (which need
        # only wv and 128 columns of xT per tile) start as early as possible.
        # Issue is round-robined over five engine queues: a single queue's
        # DMA_DIRECT2D issue cost (~600ns each) would serialize the 44 input
        # DMAs to ~26us, which stalls the chunk-1 projections.
        dma_qs = [nc.sync, nc.scalar, nc.gpsimd]
        dma_i = [0]

        def dma_in(dst, src):
            dma_qs[dma_i[0] % len(dma_qs)].dma_start(dst, src)
            dma_i[0] += 1

        wv = [const.tile([128, DH], BF16, name=f"wv{k}", tag=f"wv{k}") for k in range(8)]
        wv_v = wv_d.ap().rearrange("(t p) n -> t p n", p=128)
        xt = [const.tile([128, N], BF16, name=f"xt{k}", tag=f"xt{k}") for k in range(8)]
        xt_v = xt_d.ap().rearrange("(t p) n -> t p n", p=128)
        for k in range(8):
            dma_in(wv[k][:], wv_v[k])
            dma_in(xt[k][:, 0:512], xt_v[k][:, 0:512])
        wq = [const.tile([128, DH], BF16, name=f"wq{k}", tag=f"wq{k}") for k in range(8)]
        wq_v = wq_d.ap().rearrange("(t p) n -> t p n", p=128)
        for k in range(8):
            dma_in(wq[k][:], wq_v[k])
        wk = [const.tile([128, DH], BF16, name=f"wk{k}", tag=f"wk{k}") for k in range(8)]
        wk_v = wk_d.ap().rearrange("(t p) n -> t p n", p=128)
        for k in range(8):
            dma_in(wk[k][:], wk_v[k])
        for tc_i in range(1, 4):
            for k in range(8):
                csl = slice(tc_i * 512, (tc_i + 1) * 512)
                dma_in(xt[k][:, csl], xt_v[k][:, csl])
        wo = [const.tile([128, D], BF16, name=f"wo{k}", tag=f"wo{k}") for k in range(4)]
        wo_v = wo_d.ap().rearrange("(t p) n -> t p n", p=128)
        for k in range(4):
            dma_in(wo[k][:], wo_v[k])

        # ---- persistent intermediates ----
        qt = [const.tile([128, N], BF16, name=f"qt{k}", tag=f"qt{k}") for k in range(N_DT)]
        kt = [const.tile([128, N], BF16, name=f"kt{k}", tag=f"kt{k}") for k in range(N_DT)]
        # V' per token tile: 4 head-pair groups of [V_even(64) | 1 | V_odd(64) | 1]
        vp = [const.tile([128, 520], BF16, name=f"vp{k}", tag=f"vp{k}") for k in range(N_TT)]
        cxt = [const.tile([128, N], BF16, name=f"cxt{k}", tag=f"cxt{k}") for k in range(N_DT)]

        # ones columns of V' (offsets 64 + 65*k cover both ones cols of each pair)
        for t in range(N_TT):
            nc.vector.memset(vp[t][:, 64:520:65], 1.0)

        # ones row for the rank-1 denominator-broadcast matmul
        ones1 = const.tile([1, 64], BF16, name="ones1", tag="ones1")
        nc.vector.memset(ones1[:], 1.0)



        # ---- projection / out-proj units (filler for ACT-paced gaps) ----
        # each unit is ~4 matmuls (~850ns of PE time); state dict carries the
        # psum tile between the two halves of an 8-matmul accumulation.
        def v_units(tc_i, tt_local):
            # V projection for token tile tt = tc_i*4 + tt_local
            tt = tc_i * 4 + tt_local
            tsl = slice(tt * 128, (tt + 1) * 128)
            st = {}

            def a():
                st["pv"] = pop.tile([128, 512], F32, name="pv", tag="po")
                for k in range(4):
                    nc.tensor.matmul(
                        st["pv"][:], xt[k][:, tsl], wv[k][:, 0:DH],
                        start=(k == 0), stop=False,
                    )

            def b():
                pv = st["pv"]
                for k in range(4, 8):
                    nc.tensor.matmul(
                        pv[:], xt[k][:, tsl], wv[k][:, 0:DH],
                        start=False, stop=(k == 7),
                    )
                pv_g = pv.rearrange("p (g c) -> p g c", c=128)
                vp_g = vp[tt].rearrange("p (g c) -> p g c", c=130)
                nc.vector.tensor_copy(vp_g[:, :, 0:64], pv_g[:, :, 0:64])
                nc.vector.tensor_copy(vp_g[:, :, 65:129], pv_g[:, :, 64:128])

            return [(860, a), (860, b)]

        def qk_units(tc_i, dt):
            csl = slice(tc_i * 512, (tc_i + 1) * 512)
            dsl = slice(dt * 128, (dt + 1) * 128)
            st = {}

            def qa():
                st["pq"] = pop.tile([128, 512], F32, name="pq", tag="po")
                for k in range(4):
                    nc.tensor.matmul(
                        st["pq"][:], wq[k][:, dsl], xt[k][:, csl],
                        start=(k == 0), stop=False,
                    )

            def qb():
                pq = st["pq"]
                for k in range(4, 8):
                    nc.tensor.matmul(
                        pq[:], wq[k][:, dsl], xt[k][:, csl],
                        start=False, stop=(k == 7),
                    )
                nc.vector.tensor_copy(qt[dt][:, csl], pq[:])

            def ka():
                st["pk"] = pop.tile([128, 512], F32, name="pk", tag="po")
                for k in range(4):
                    nc.tensor.matmul(
                        st["pk"][:], wk[k][:, dsl], xt[k][:, csl],
                        start=(k == 0), stop=False,
                    )

            def kb():
                pk = st["pk"]
                for k in range(4, 8):
                    nc.tensor.matmul(
                        pk[:], wk[k][:, dsl], xt[k][:, csl],
                        start=False, stop=(k == 7),
                    )
                nc.vector.tensor_copy(kt[dt][:, csl], pk[:])

            return [(860, qa), (860, qb), (860, ka), (860, kb)]

        def outproj_unit(qc, u, tags=("po",)):
            tti, nck = u // 2, u % 2
            tt = qc * 4 + tti
            tsl = slice(tt * 128, (tt + 1) * 128)
            nsl = slice(nck * 512, (nck + 1) * 512)
            tag = tags[u % len(tags)]

            def f():
                if tag == "ps":
                    po = ps.tile([128, 512], F32, name="po", tag="ps")
                elif tag == "ctx":
                    po = ctxp.tile([128, 512], F32, name="po", tag="ctx")
                else:
                    po = pop.tile([128, 512], F32, name="po", tag="po")
                for dt2 in range(N_DT):
                    nc.tensor.matmul(
                        po[:], cxt[dt2][:, tsl], wo[dt2][:, nsl],
                        start=(dt2 == 0), stop=(dt2 == 3),
                    )
                ob = outp.tile([128, 512], F32, name="ob", tag="ob")
                nc.vector.tensor_copy(ob[:], po[:])
                nc.sync.dma_start(out_d.ap()[tsl, nsl], ob[:])

            return (900, f)

        # ---- filler queue with deadline-based flushing ----
        # entries: [deadline_key, cost_ns, fn]; deadline_key = (qc, dt) before
        # whose attention block the unit MUST have been emitted (proj deps);
        # (99, 99) = no deadline (out-proj units).
        work = []

        def pump(ns):
            # emit filler units from the front until ~ns of PE time is covered
            spent = 0
            while work and spent < ns:
                _, cost, fn = work.pop(0)
                fn()
                spent += cost
            return spent

        def flush(qc, dt):
            i = 0
            while i < len(work):
                if work[i][0] <= (qc, dt):
                    _, _, fn = work.pop(i)
                    fn()
                else:
                    i += 1

        # ---- attention block (qc, dt): software-pipelined kv loop ----
        deficit = [0.0]

        def attn_block(qc, dt):
            qsl = slice(qc * 512, (qc + 1) * 512)
            ea = slice(0, 64)     # even head of the pair: partitions 0:64
            eb = slice(64, 128)   # odd head: partitions 64:128
            va = slice(dt * 130, dt * 130 + 65)        # [V_even | 1]
            vb = slice(dt * 130 + 65, dt * 130 + 130)  # [V_odd | 1]
            ca = ctxp.tile([65, 512], F32, name="ca", tag="ctx")
            cb = ctxp.tile([65, 512], F32, name="cb", tag="ctx")
            nkt = 4 * (qc + 1)
            # diagonal kv-tiles first: their longer exp->mask->ctx chain
            # then overlaps the independent (unmasked) off-diagonal tiles.
            order = list(reversed(range(nkt)))
            pend = []

            def emit_ctx(p):
                ktl_, qoff_, at_, first_, last_ = p
                nc.tensor.matmul(
                    ca[:, qoff_:512], vp[ktl_][:, va], at_[:, qoff_:512],
                    start=first_, stop=last_,
                )
                nc.tensor.matmul(
                    cb[:, qoff_:512], vp[ktl_][:, vb], at_[:, 512 + qoff_:1024],
                    start=first_, stop=last_,
                )

            for i, ktl in enumerate(order):
                ksl = slice(ktl * 128, ktl * 128 + 128)
                j = ktl - 4 * qc
                qoff = 128 * j if j > 0 else 0
                nw = 512 - qoff
                qn = slice(qc * 512 + qoff, (qc + 1) * 512)
                s = ps.tile([128, 1024], F32, name="s", tag="ps")
                nc.tensor.matmul(s[:, qoff:512], kt[dt][ea, ksl], qt[dt][ea, qn], start=True, stop=True)
                nc.tensor.matmul(s[:, 512 + qoff:1024], kt[dt][eb, ksl], qt[dt][eb, qn], start=True, stop=True)
                at = attn_pool.tile([128, 1024], BF16, name="at", tag="attn")
                s3 = s.rearrange("p (o q) -> p o q", o=2)[:, :, qoff:512]
                at3 = at.rearrange("p (o q) -> p o q", o=2)[:, :, qoff:512]
                nc.scalar.activation(at3, s3, AF.Exp, scale=0.125)
                if j >= 0:
                    # diagonal: zero attn where kv > q (pure triangle after
                    # the qoff shift; both halves = same kv-tile)
                    nc.gpsimd.affine_select(
                        at3,
                        at3,
                        pattern=[[0, 2], [1, nw]],
                        compare_op=mybir.AluOpType.is_ge,
                        fill=0.0,
                        base=0,
                        channel_multiplier=-1,
                    )
                # depth-2 software pipeline: emit the ctx of tile i-2, so the
                # block-boundary ctx never races the previous block's
                # normalize chain (which frees the ca/cb psum slots)
                if len(pend) >= 2:
                    emit_ctx(pend.pop(0))
                # deficit-paced filler: ACT time minus attention PE time
                # (constants measured from traces: score pair ~0.64nw incl.
                # partial overlap, ctx ~0.9nw incl. exposed LDWEIGHTS, plus
                # ~100ns/tile of fixed dispatch cost). no pumping during
                # qc=0: the queued projection units need xT chunks still in
                # flight, and an emitted-but-unready matmul blocks the whole
                # PE stream.
                deficit[0] += (2 * nw + 352) / 1.2 + 80 - (0.63 * nw + 0.834 * nw)
                if deficit[0] > 0 and qc > 0:
                    deficit[0] -= pump(deficit[0])
                pend.append((ktl, qoff, at, i == 0, i == nkt - 1))
            for p in pend:
                emit_ctx(p)

            # normalize and copy back to SBUF (bf16)
            # custom-DVE ops don't handle partition-offset inputs; stage the
            # denom row at partition 0 first (builtin copy does remap lanes)
            da = small.tile([1, 512], F32, name="da", tag="d")
            db = small.tile([1, 512], F32, name="db", tag="d")
            nc.vector.tensor_copy(da[:], ca[64:65, :])
            nc.vector.tensor_copy(db[:], cb[64:65, :])
            ra = small.tile([1, 512], F32, name="ra", tag="r")
            rb = small.tile([1, 512], F32, name="rb", tag="r")
            nc.vector.reciprocal_approx_fast(ra[:], da[:])
            nc.vector.reciprocal_approx_fast(rb[:], db[:])
            rab = small.tile([1, 512], BF16, name="rab", tag="rc")
            rbb2 = small.tile([1, 512], BF16, name="rbb2", tag="rc")
            nc.vector.tensor_copy(rab[:], ra[:])
            nc.vector.tensor_copy(rbb2[:], rb[:])
            # broadcast the recips across 64 partitions with rank-1 matmuls
            # (ones[1,64].T @ recip[1,512]); keeps gpsimd free for the masks
            rp = pop.tile([128, 512], F32, name="rp", tag="po")
            nc.tensor.matmul(rp[0:64, :], ones1[:], rab[:], start=True, stop=True)
            nc.tensor.matmul(rp[64:128, :], ones1[:], rbb2[:], start=True, stop=True)
            rs = small.tile([128, 512], F32, name="rs", tag="rs")
            nc.vector.tensor_copy(rs[:], rp[:])
            nc.vector.tensor_mul(cxt[dt][0:64, qsl], ca[0:64, :], rs[0:64, :])
            tmpb = small.tile([64, 512], BF16, name="tmpb", tag="tmp")
            nc.vector.tensor_mul(tmpb[:], cb[0:64, :], rs[64:128, :])
            # partition shift 0:64 -> 64:128 (engines are lane-locked; DMA is not)
            nc.sync.dma_start(cxt[dt][64:128, qsl], tmpb[:])

        # ---- head phase: V projections for chunk 0, then q/k for dt=0 ----
        for tt_local in range(4):
            for _, fn in v_units(0, tt_local):
                fn()
        for _, fn in qk_units(0, 0):
            fn()
        # queue the rest with deadlines
        for dt in range(1, 4):
            for c, fn in qk_units(0, dt):
                work.append([(0, dt), c, fn])
        for tc_i in range(1, 4):
            for tt_local in range(4):
                for c, fn in v_units(tc_i, tt_local):
                    work.append([(tc_i, 0), c, fn])
            for dt in range(4):
                for c, fn in qk_units(tc_i, dt):
                    work.append([(tc_i, dt), c, fn])

        # ---- main loop ----
        for qc in range(N_QC):
            for dt in range(N_DT):
                flush(qc, dt)
                attn_block(qc, dt)
            if qc < N_QC - 1:
                for u in range(8):
                    c, fn = outproj_unit(qc, u)
                    work.append([(99, 99), c, fn])
        # tail: remaining filler + final out-proj across all psum banks
        while work:
            _, _, fn = work.pop(0)
            fn()
        for u in range(8):
            _, fn = outproj_unit(N_QC - 1, u, tags=("po", "ps", "ctx", "ps"))
            fn()


def build_bass():
    nc = bacc.Bacc("TRN2", target_bir_lowering=False, debug=False, num_devices=N_CORES)
    xt_d = nc.dram_tensor("xt", (D, N), BF16, kind="ExternalInput")
    wq_d = nc.dram_tensor("wq", (D, DH), BF16, kind="ExternalInput")
    wk_d = nc.dram_tensor("wk", (D, DH), BF16, kind="ExternalInput")
    wv_d = nc.dram_tensor("wv", (D, DH), BF16, kind="ExternalInput")
    wo_d = nc.dram_tensor("wo", (DH, D), BF16, kind="ExternalInput")
    out_d = nc.dram_tensor("out", (N, D), F32, kind="ExternalOutput")
    with tile.TileContext(nc) as tc:
        _emit(nc, tc, xt_d, wq_d, wk_d, wv_d, wo_d, out_d)
    nc.compile()
    return nc


_NC = None


def _get_nc():
    global _NC
    if _NC is None:
        _NC = build_bass()
    return _NC


def make_in_maps(x, Wq, Wk, Wv, Wo):
    bf = ml_dtypes.bfloat16
    in_maps = []
    for c in range(N_CORES):
        b, g = c // 2, c % 2
        gs = slice(g * DH, (g + 1) * DH)
        in_maps.append(
            {
                "xt": np.ascontiguousarray(x[b].T).astype(bf),
                "wq": np.ascontiguousarray(Wq[:, gs]).astype(bf),
                "wk": np.ascontiguousarray(Wk[:, gs]).astype(bf),
                "wv": np.ascontiguousarray(Wv[:, gs]).astype(bf),
                "wo": np.ascontiguousarray(Wo[gs, :]).astype(bf),
            }
        )
    return in_maps


def kernel(x, Wq, Wk, Wv, Wo, bo, _trace=False):
    x = np.asarray(x, dtype=np.float32)
    nc = _get_nc()
    in_maps = make_in_maps(x, Wq, Wk, Wv, Wo)
    res = bass_utils.run_bass_kernel_spmd(
        nc, in_maps, core_ids=list(range(N_CORES)), trace=_trace
    )
    out = np.empty((B, N, D), dtype=np.float32)
    bo32 = np.asarray(bo, dtype=np.float32)
    for b in range(B):
        out[b] = res.results[2 * b]["out"] + res.results[2 * b + 1]["out"] + bo32
    if _trace:
        return out, res
    return out


# revision 23
# speedup vs baseline: 1.2399x; 1.2399x over previous
"""Multi-head causal attention (b=4, n=2048, d=1024, h=16) on 8 TRN2 cores.

Sharding: core c = (batch b = c//2, head-group g = c%2); each head-group is 8
heads = 512 of the 1024 model dims. QKV weights column-sharded, Wo row-sharded;
host sums the two head-group partial outputs per batch and adds the bias.

Per-core layout: everything is kept in "transposed" orientation so each
matmul feeds the next without any on-chip transposes:
  QT/KT [dout, tok] = W.T @ xT        (lhsT = W as stored, rhs = xT)
  scoresT [kv, q]   = KT_h.T @ QT_h   (contraction over head-dim, K=64,
                                       2 heads row-packed in the PE array)
  attnT             = exp(scoresT/8)  (ACT, PSUM->SBUF bf16; no max-subtraction:
                                       |scores/8| < ~2 for this input dist)
  causal mask       = DVE multiply by a constant lower-triangular bf16 tile
  ctxT [hd, q]      = V_h'.T @ attnT  (V_h' has a ones column appended, so PSUM
                                       row 64 accumulates the softmax denom)
  normalize         = DVE reciprocal_approx + gpsimd partition-broadcast +
                      DVE multiply at PSUM->SBUF copyback
  out [tok, dout]   = ctxT.T @ Wo     (partial over this head-group's 512 dims)

v2 emission schedule (vs ~303us baseline):
- software-pipelined kv loop: the ctx matmuls of kv-tile i-1 are emitted after
  the score-pair of tile i, and a deficit-paced filler queue (projection and
  out-proj units, each ~4 matmuls) is drained inside the loop so the PE always
  has independent work during the ACT-paced exp gaps.
- deficit model: ACT cost (2nw+352)/1.2 ns per tile vs PE cost ~0.63nw (score
  pair, measured 322ns@nw=512) + 0.83nw (ctx pair); the shortfall is pumped
  from the filler queue.
- V projections first + finer DMA interleave so attention starts ~11us in
  instead of ~25us.
- causal masks moved from gpsimd affine_select to DVE tri-tile multiplies
  (shorter latency, keeps gpsimd free for the normalize broadcasts).
- out-proj units are appended to the END of the filler queue as their chunk
  completes, so they back-fill the ACT-heavy final query chunk where no
  projection filler remains.
"""

import sys

if "/opt/trn_rl_repo" not in sys.path:
    sys.path.insert(0, "/opt/trn_rl_repo")

import numpy as np
import ml_dtypes

import concourse.bacc as bacc
import concourse.mybir as mybir
import concourse.tile as tile
from concourse import bass_utils

N_CORES = 8
B = 4          # batch
N = 2048       # sequence length
D = 1024       # model dim
H = 16         # total heads
HD = 64        # head dim
HH = 8         # heads per core
DH = 512       # model dims per core (HH * HD)
N_DT = 4       # 128-row d-tiles of DH (one head pair each)
N_QC = 4       # 512-wide query chunks
N_KT = 16      # 128-wide kv token tiles
N_TT = 16      # 128-wide token tiles
BF16 = mybir.dt.bfloat16
F32 = mybir.dt.float32
AF = mybir.ActivationFunctionType


def _emit(nc, tc, xt_d, wq_d, wk_d, wv_d, wo_d, out_d):
    import contextlib

    ctx = contextlib.ExitStack()
    with ctx:
        const = ctx.enter_context(tc.tile_pool(name="const", bufs=1))
        ps = ctx.enter_context(tc.tile_pool(name="ps", bufs=2, space="PSUM"))
        ctxp = ctx.enter_context(tc.tile_pool(name="ctxp", bufs=2, space="PSUM"))
        pop = ctx.enter_context(tc.tile_pool(name="pop", bufs=2, space="PSUM"))
        attn_pool = ctx.enter_context(tc.tile_pool(name="attn", bufs=12))
        small = ctx.enter_context(tc.tile_pool(name="small", bufs=3))
        outp = ctx.enter_context(tc.tile_pool(name="outp", bufs=4))

        # ---- input DMAs ----
        # wv + the first xT token-chunk first: the V projections (which need
        # only wv and 128 columns of xT per tile) start as early as possible.
        # Issue is round-robined over five engine queues: a single queue's
        # DMA_DIRECT2D issue cost (~600ns each) would serialize the 44 input
        # DMAs to ~26us, which stalls the chunk-1 projections.
        dma_qs = [nc.sync, nc.scalar, nc.gpsimd]
        dma_i = [0]

        def dma_in(dst, src):
            dma_qs[dma_i[0] % len(dma_qs)].dma_start(dst, src)
            dma_i[0] += 1

        wv = [const.tile([128, DH], BF16, name=f"wv{k}", tag=f"wv{k}") for k in range(8)]
        wv_v = wv_d.ap().rearrange("(t p) n -> t p n", p=128)
        xt = [const.tile([128, N], BF16, name=f"xt{k}", tag=f"xt{k}") for k in range(8)]
        xt_v = xt_d.ap().rearrange("(t p) n -> t p n", p=128)
        for k in range(8):
            dma_in(wv[k][:], wv_v[k])
            dma_in(xt[k][:, 0:512], xt_v[k][:, 0:512])
        wq = [const.tile([128, DH], BF16, name=f"wq{k}", tag=f"wq{k}") for k in range(8)]
        wq_v = wq_d.ap().rearrange("(t p) n -> t p n", p=128)
        for k in range(8):
            dma_in(wq[k][:], wq_v[k])
        wk = [const.tile([128, DH], BF16, name=f"wk{k}", tag=f"wk{k}") for k in range(8)]
        wk_v = wk_d.ap().rearrange("(t p) n -> t p n", p=128)
        for k in range(8):
            dma_in(wk[k][:], wk_v[k])
        for tc_i in range(1, 4):
            for k in range(8):
                csl = slice(tc_i * 512, (tc_i + 1) * 512)
                dma_in(xt[k][:, csl], xt_v[k][:, csl])
        wo = [const.tile([128, D], BF16, name=f"wo{k}", tag=f"wo{k}") for k in range(4)]
        wo_v = wo_d.ap().rearrange("(t p) n -> t p n", p=128)
        for k in range(4):
            dma_in(wo[k][:], wo_v[k])

        # ---- persistent intermediates ----
        qt = [const.tile([128, N], BF16, name=f"qt{k}", tag=f"qt{k}") for k in range(N_DT)]
        kt = [const.tile([128, N], BF16, name=f"kt{k}", tag=f"kt{k}") for k in range(N_DT)]
        # V' per token tile: 4 head-pair groups of [V_even(64) | 1 | V_odd(64) | 1]
        vp = [const.tile([128, 520], BF16, name=f"vp{k}", tag=f"vp{k}") for k in range(N_TT)]
        cxt = [const.tile([128, N], BF16, name=f"cxt{k}", tag=f"cxt{k}") for k in range(N_DT)]

        # ones columns of V' (offsets 64 + 65*k cover both ones cols of each pair)
        for t in range(N_TT):
            nc.vector.memset(vp[t][:, 64:520:65], 1.0)





        # ---- projection / out-proj units (filler for ACT-paced gaps) ----
        # each unit is ~4 matmuls (~850ns of PE time); state dict carries the
        # psum tile between the two halves of an 8-matmul accumulation.
        def v_units(tc_i, tt_local):
            # V projection for token tile tt = tc_i*4 + tt_local
            tt = tc_i * 4 + tt_local
            tsl = slice(tt * 128, (tt + 1) * 128)
            st = {}

            def a():
                st["pv"] = pop.tile([128, 512], F32, name="pv", tag="po")
                for k in range(4):
                    nc.tensor.matmul(
                        st["pv"][:], xt[k][:, tsl], wv[k][:, 0:DH],
                        start=(k == 0), stop=False,
                    )

            def b():
                pv = st["pv"]
                for k in range(4, 8):
                    nc.tensor.matmul(
                        pv[:], xt[k][:, tsl], wv[k][:, 0:DH],
                        start=False, stop=(k == 7),
                    )
                pv_g = pv.rearrange("p (g c) -> p g c", c=128)
                vp_g = vp[tt].rearrange("p (g c) -> p g c", c=130)
                nc.vector.tensor_copy(vp_g[:, :, 0:64], pv_g[:, :, 0:64])
                nc.vector.tensor_copy(vp_g[:, :, 65:129], pv_g[:, :, 64:128])

            return [(860, a), (860, b)]

        def qk_units(tc_i, dt):
            csl = slice(tc_i * 512, (tc_i + 1) * 512)
            dsl = slice(dt * 128, (dt + 1) * 128)
            st = {}

            def qa():
                st["pq"] = pop.tile([128, 512], F32, name="pq", tag="po")
                for k in range(4):
                    nc.tensor.matmul(
                        st["pq"][:], wq[k][:, dsl], xt[k][:, csl],
                        start=(k == 0), stop=False,
                    )

            def qb():
                pq = st["pq"]
                for k in range(4, 8):
                    nc.tensor.matmul(
                        pq[:], wq[k][:, dsl], xt[k][:, csl],
                        start=False, stop=(k == 7),
                    )
                nc.vector.tensor_copy(qt[dt][:, csl], pq[:])

            def ka():
                st["pk"] = pop.tile([128, 512], F32, name="pk", tag="po")
                for k in range(4):
                    nc.tensor.matmul(
                        st["pk"][:], wk[k][:, dsl], xt[k][:, csl],
                        start=(k == 0), stop=False,
                    )

            def kb():
                pk = st["pk"]
                for k in range(4, 8):
                    nc.tensor.matmul(
                        pk[:], wk[k][:, dsl], xt[k][:, csl],
                        start=False, stop=(k == 7),
                    )
                nc.vector.tensor_copy(kt[dt][:, csl], pk[:])

            return [(860, qa), (860, qb), (860, ka), (860, kb)]

        def outproj_unit(qc, u, tags=("po",)):
            tti, nck = u // 2, u % 2
            tt = qc * 4 + tti
            tsl = slice(tt * 128, (tt + 1) * 128)
            nsl = slice(nck * 512, (nck + 1) * 512)
            tag = tags[u % len(tags)]

            def f():
                if tag == "ps":
                    po = ps.tile([128, 512], F32, name="po", tag="ps")
                elif tag == "ctx":
                    po = ctxp.tile([128, 512], F32, name="po", tag="ctx")
                else:
                    po = pop.tile([128, 512], F32, name="po", tag="po")
                for dt2 in range(N_DT):
                    nc.tensor.matmul(
                        po[:], cxt[dt2][:, tsl], wo[dt2][:, nsl],
                        start=(dt2 == 0), stop=(dt2 == 3),
                    )
                ob = outp.tile([128, 512], F32, name="ob", tag="ob")
                nc.vector.tensor_copy(ob[:], po[:])
                nc.sync.dma_start(out_d.ap()[tsl, nsl], ob[:])

            return (900, f)

        # ---- filler queue with deadline-based flushing ----
        # entries: [deadline_key, cost_ns, fn]; deadline_key = (qc, dt) before
        # whose attention block the unit MUST have been emitted (proj deps);
        # (99, 99) = no deadline (out-proj units).
        work = []

        def pump(ns):
            # emit filler units from the front until ~ns of PE time is covered
            spent = 0
            while work and spent < ns:
                _, cost, fn = work.pop(0)
                fn()
                spent += cost
            return spent

        def flush(qc, dt):
            i = 0
            while i < len(work):
                if work[i][0] <= (qc, dt):
                    _, _, fn = work.pop(i)
                    fn()
                else:
                    i += 1

        # ---- attention block (qc, dt): software-pipelined kv loop ----
        deficit = [0.0]

        def attn_block(qc, dt):
            qsl = slice(qc * 512, (qc + 1) * 512)
            ea = slice(0, 64)     # even head of the pair: partitions 0:64
            eb = slice(64, 128)   # odd head: partitions 64:128
            va = slice(dt * 130, dt * 130 + 65)        # [V_even | 1]
            vb = slice(dt * 130 + 65, dt * 130 + 130)  # [V_odd | 1]
            ca = ctxp.tile([65, 512], F32, name="ca", tag="ctx")
            cb = ctxp.tile([65, 512], F32, name="cb", tag="ctx")
            nkt = 4 * (qc + 1)
            # diagonal kv-tiles first: their longer exp->mask->ctx chain
            # then overlaps the independent (unmasked) off-diagonal tiles.
            order = list(reversed(range(nkt)))
            pend = []

            def emit_ctx(p):
                ktl_, qoff_, at_, first_, last_ = p
                nc.tensor.matmul(
                    ca[:, qoff_:512], vp[ktl_][:, va], at_[:, qoff_:512],
                    start=first_, stop=last_,
                )
                nc.tensor.matmul(
                    cb[:, qoff_:512], vp[ktl_][:, vb], at_[:, 512 + qoff_:1024],
                    start=first_, stop=last_,
                )

            for i, ktl in enumerate(order):
                ksl = slice(ktl * 128, ktl * 128 + 128)
                j = ktl - 4 * qc
                qoff = 128 * j if j > 0 else 0
                nw = 512 - qoff
                qn = slice(qc * 512 + qoff, (qc + 1) * 512)
                s = ps.tile([128, 1024], F32, name="s", tag="ps")
                nc.tensor.matmul(s[:, qoff:512], kt[dt][ea, ksl], qt[dt][ea, qn], start=True, stop=True)
                nc.tensor.matmul(s[:, 512 + qoff:1024], kt[dt][eb, ksl], qt[dt][eb, qn], start=True, stop=True)
                at = attn_pool.tile([128, 1024], BF16, name="at", tag="attn")
                s3 = s.rearrange("p (o q) -> p o q", o=2)[:, :, qoff:512]
                at3 = at.rearrange("p (o q) -> p o q", o=2)[:, :, qoff:512]
                nc.scalar.activation(at3, s3, AF.Exp, scale=0.125)
                if j >= 0:
                    # diagonal: zero attn where kv > q (pure triangle after
                    # the qoff shift; both halves = same kv-tile)
                    nc.gpsimd.affine_select(
                        at3,
                        at3,
                        pattern=[[0, 2], [1, nw]],
                        compare_op=mybir.AluOpType.is_ge,
                        fill=0.0,
                        base=0,
                        channel_multiplier=-1,
                    )
                # depth-2 software pipeline: emit the ctx of tile i-2, so the
                # block-boundary ctx never races the previous block's
                # normalize chain (which frees the ca/cb psum slots)
                if len(pend) >= 2:
                    emit_ctx(pend.pop(0))
                # deficit-paced filler: ACT time minus attention PE time
                # (constants measured from traces: score pair ~0.64nw incl.
                # partial overlap, ctx ~0.9nw incl. exposed LDWEIGHTS, plus
                # ~100ns/tile of fixed dispatch cost). no pumping during
                # qc=0: the queued projection units need xT chunks still in
                # flight, and an emitted-but-unready matmul blocks the whole
                # PE stream.
                deficit[0] += (2 * nw + 352) / 1.2 + 80 - (0.63 * nw + 0.834 * nw)
                if deficit[0] > 0 and (qc > 0 or dt == 3):
                    deficit[0] -= pump(deficit[0])
                pend.append((ktl, qoff, at, i == 0, i == nkt - 1))
            for p in pend:
                emit_ctx(p)

            # normalize and copy back to SBUF (bf16)
            # custom-DVE ops don't handle partition-offset inputs; stage the
            # denom row at partition 0 first (builtin copy does remap lanes)
            da = small.tile([1, 512], F32, name="da", tag="d")
            db = small.tile([1, 512], F32, name="db", tag="d")
            nc.vector.tensor_copy(da[:], ca[64:65, :])
            nc.vector.tensor_copy(db[:], cb[64:65, :])
            ra = small.tile([1, 512], F32, name="ra", tag="r")
            rb = small.tile([1, 512], F32, name="rb", tag="r")
            nc.vector.reciprocal_approx_fast(ra[:], da[:])
            nc.vector.reciprocal_approx_fast(rb[:], db[:])
            rba = small.tile([64, 512], F32, name="rba", tag="rb")
            rbb = small.tile([64, 512], F32, name="rbb", tag="rb")
            nc.gpsimd.partition_broadcast(rba[:], ra[:])
            nc.gpsimd.partition_broadcast(rbb[:], rb[:])
            nc.vector.tensor_mul(cxt[dt][0:64, qsl], ca[0:64, :], rba[:])
            tmpb = small.tile([64, 512], BF16, name="tmpb", tag="tmp")
            nc.vector.tensor_mul(tmpb[:], cb[0:64, :], rbb[:])
            # partition shift 0:64 -> 64:128 (engines are lane-locked; DMA is not)
            nc.sync.dma_start(cxt[dt][64:128, qsl], tmpb[:])

        # ---- head phase: V projections for chunk 0, then q/k for dt=0 ----
        for tt_local in range(4):
            for _, fn in v_units(0, tt_local):
                fn()
        for _, fn in qk_units(0, 0):
            fn()
        # queue the rest with deadlines
        for dt in range(1, 4):
            for c, fn in qk_units(0, dt):
                work.append([(0, dt), c, fn])
        for tc_i in range(1, 4):
            for tt_local in range(4):
                for c, fn in v_units(tc_i, tt_local):
                    work.append([(tc_i, 0), c, fn])
            for dt in range(4):
                for c, fn in qk_units(tc_i, dt):
                    work.append([(tc_i, dt), c, fn])

        # ---- main loop ----
        for qc in range(N_QC):
            for dt in range(N_DT):
                flush(qc, dt)
                attn_block(qc, dt)
            if qc < N_QC - 1:
                for u in range(8):
                    c, fn = outproj_unit(qc, u)
                    work.append([(99, 99), c, fn])
        # tail: remaining filler + final out-proj across all psum banks
        while work:
            _, _, fn = work.pop(0)
            fn()
        for u in range(8):
            _, fn = outproj_unit(N_QC - 1, u, tags=("po", "po", "ps", "ps", "ctx", "ctx"))
            fn()


def build_bass():
    nc = bacc.Bacc("TRN2", target_bir_lowering=False, debug=False, num_devices=N_CORES)
    xt_d = nc.dram_tensor("xt", (D, N), BF16, kind="ExternalInput")
    wq_d = nc.dram_tensor("wq", (D, DH), BF16, kind="ExternalInput")
    wk_d = nc.dram_tensor("wk", (D, DH), BF16, kind="ExternalInput")
    wv_d = nc.dram_tensor("wv", (D, DH), BF16, kind="ExternalInput")
    wo_d = nc.dram_tensor("wo", (DH, D), BF16, kind="ExternalInput")
    out_d = nc.dram_tensor("out", (N, D), F32, kind="ExternalOutput")
    with tile.TileContext(nc) as tc:
        _emit(nc, tc, xt_d, wq_d, wk_d, wv_d, wo_d, out_d)
    nc.compile()
    return nc


_NC = None


def _get_nc():
    global _NC
    if _NC is None:
        _NC = build_bass()
    return _NC


def make_in_maps(x, Wq, Wk, Wv, Wo):
    bf = ml_dtypes.bfloat16
    in_maps = []
    for c in range(N_CORES):
        b, g = c // 2, c % 2
        gs = slice(g * DH, (g + 1) * DH)
        in_maps.append(
            {
                "xt": np.ascontiguousarray(x[b].T).astype(bf),
                "wq": np.ascontiguousarray(Wq[:, gs]).astype(bf),
                "wk": np.ascontiguousarray(Wk[:, gs]).astype(bf),
                "wv": np.ascontiguousarray(Wv[:, gs]).astype(bf),
                "wo": np.ascontiguousarray(Wo[gs, :]).astype(bf),
            }
        )
    return in_maps


def kernel(x, Wq, Wk, Wv, Wo, bo, _trace=False):
    x = np.asarray(x, dtype=np.float32)
    nc = _get_nc()
    in_maps = make_in_maps(x, Wq, Wk, Wv, Wo)
    res = bass_utils.run_bass_kernel_spmd(
        nc, in_maps, core_ids=list(range(N_CORES)), trace=_trace
    )
    out = np.empty((B, N, D), dtype=np.float32)
    bo32 = np.asarray(bo, dtype=np.float32)
    for b in range(B):
        out[b] = res.results[2 * b]["out"] + res.results[2 * b + 1]["out"] + bo32
    if _trace:
        return out, res
    return out
